# revision 12
# baseline (speedup 1.0000x reference)
"""BiRNN (Bowman SNLI) Trainium2 kernel, quad-LSTM column-tiled design.

Sharding: 8 cores = 8 batch slices of 32 rows. Each core runs ALL FOUR
LSTMs (fw_p, bw_p, fw_h, bw_h) for its 32 rows, partition-stacked: job j
occupies batch columns [32j, 32j+32) of every stationary operand and PSUM
rows [32j, 32j+32). Each matmul slot issues 4 concurrent column-tiled MMs
(M=32 at tile_position (0, 32j)) with per-job weights as the moving
operand — the PE runs them in parallel, so the 4 LSTMs cost one LSTM's
matmul time. The elementwise tail (gates/c/h) spans all 4 jobs in single
[128, *] ops, quartering its per-LSTM-step cost vs one-job-per-core.
h^T for step t+1 comes from 4 PE transposes of the stacked h(t), emitted
between step t+1's x-inject MMs and its h-MMs so the tail overlaps the
x-block. No collectives: after the scan each core holds all four final
cell states for its 32 rows and runs the MLP locally (batch-32,
activations-stationary layout with PE transposes between layers).
"""
import numpy as np
import ml_dtypes

# Harness-visible constants
B, T, E, H, F = 256, 128, 300, 512, 1024
BC = 32           # batch rows per core
N_CORES = 8
EP = 384          # padded x feature dim (300 x + 1 bias + pad)
KX = 3            # x stationary chunks (last has 45 valid rows)
KH = 4            # h stationary chunks
NJ = 4            # LSTM jobs per core

_cache = {}


def _apply_tile_patch():
    """walrus here allows ONE semaphore wait per instruction; Tile's tail
    drain (and occasionally other instructions) get more. Split extra waits
    onto same-engine NoOp carriers inserted immediately before."""
    import concourse.tile as tile
    import concourse.mybir as mybir
    from concourse.tile import ScopedClock

    if getattr(tile.TileContext, "_multiwait_patched", False):
        return

    def split_multiwait(nc):
        for f in nc.m.functions:
            for bb in f.blocks:
                insts = bb.instructions
                if not any(
                    i.sync_info is not None and len(i.sync_info.on_wait) > 1
                    for i in insts
                ):
                    continue
                new = []
                for inst in insts:
                    si = inst.sync_info
                    if si is not None and len(si.on_wait) > 1:
                        waits = list(si.on_wait)
                        for w in waits[:-1]:
                            carrier = mybir.InstNoOp(
                                name=nc.get_next_instruction_name(), ins=[], outs=[]
                            )
                            carrier.engine = inst.engine
                            carrier.sync_info = mybir.SyncInfo(
                                on_wait=[w], on_update=[]
                            )
                            nc.register_instruction(carrier, overwrite=True)
                            new.append(carrier)
                        si.on_wait = [waits[-1]]
                    new.append(inst)
                bb.instructions = new

    def _patched_drain_and_barrier(self, tick_clock, wait_clock):
        nc = self.nc
        drain_inst = nc.sync.drain()
        wait_clock.add_sem_waits(
            drain_inst.ins, ScopedClock({None: tick_clock.global_clock})
        )
        nc.all_engine_barrier()
        assert self.sems is not None
        popped = nc._tile_sem_poison_stack.pop()
        assert popped is self._sem_poison
        nc.clear_and_free_semaphores(list(self.sems.allocated().values()))
        nc.all_engine_barrier()
        split_multiwait(nc)

    tile.TileContext._drain_and_barrier = _patched_drain_and_barrier
    tile.TileContext._multiwait_patched = True


def _build_nc(t_steps=T, ldt="bf16"):
    _apply_tile_patch()
    from contextlib import ExitStack
    import concourse.bass as bass
    import concourse.tile as tile
    from concourse import mybir

    f32 = mybir.dt.float32
    bf16 = mybir.dt.bfloat16
    AF = mybir.ActivationFunctionType

    nc = bass.Bass("TRN2", target_bir_lowering=False, debug=False,
                   num_devices=N_CORES)

    xt_d = nc.dram_tensor("xt", [t_steps, 128, KX * 128], bf16, kind="ExternalInput").ap()
    wl_d = nc.dram_tensor("wl", [128, NJ, KX + KH, 4 * H], bf16, kind="ExternalInput").ap()
    w1_d = nc.dram_tensor("w1", [128, 16, F], bf16, kind="ExternalInput").ap()
    w2_d = nc.dram_tensor("w2", [128, 8, F], bf16, kind="ExternalInput").ap()
    w3_d = nc.dram_tensor("w3", [128, 8, F], bf16, kind="ExternalInput").ap()
    w4_d = nc.dram_tensor("w4", [128, 8, 3], bf16, kind="ExternalInput").ap()
    b1_d = nc.dram_tensor("b1", [1, F], bf16, kind="ExternalInput").ap()
    b2_d = nc.dram_tensor("b2", [1, F], bf16, kind="ExternalInput").ap()
    b3_d = nc.dram_tensor("b3", [1, F], bf16, kind="ExternalInput").ap()
    b4_d = nc.dram_tensor("b4", [1, 3], bf16, kind="ExternalInput").ap()
    ones_d = nc.dram_tensor("ones", [1, BC], bf16, kind="ExternalInput").ap()
    idr_d = nc.dram_tensor("identr", [128, 128], bf16, kind="ExternalInput").ap()
    out_d = nc.dram_tensor("logits", [BC, 3], f32, kind="ExternalOutput").ap()

    G4 = 4 * H  # 2048

    with tile.TileContext(nc) as tc, ExitStack() as ctx:
        wp = ctx.enter_context(tc.tile_pool(name="weights", bufs=1))
        wl_sb = wp.tile([128, NJ, KX + KH, G4], bf16, tag="wl")
        w1_sb = wp.tile([128, 16, F], bf16, tag="w1")
        w2_sb = wp.tile([128, 8, F], bf16, tag="w2")
        w3_sb = wp.tile([128, 8, F], bf16, tag="w3")
        w4_sb = wp.tile([128, 8, 3], bf16, tag="w4")
        b1_sb = wp.tile([1, F], bf16, tag="b1")
        b2_sb = wp.tile([1, F], bf16, tag="b2")
        b3_sb = wp.tile([1, F], bf16, tag="b3")
        b4_sb = wp.tile([1, 3], bf16, tag="b4")
        ones_sb = wp.tile([1, BC], bf16, tag="ones")
        idr_sb = wp.tile([128, 128], bf16, tag="idr")
        # LSTM-critical loads first; one DMA per (job, chunk) so the
        # transfers spread across queues and early chunks land first
        nc.sync.dma_start(idr_sb[:], idr_d[:])
        for k in range(KX + KH):
            for j in range(NJ):
                nc.sync.dma_start(wl_sb[:, j, k, :], wl_d[:, j, k])
        _mlp_loads = [(w1_sb, w1_d), (w2_sb, w2_d), (w3_sb, w3_d),
                      (w4_sb, w4_d), (b1_sb, b1_d), (b2_sb, b2_d),
                      (b3_sb, b3_d), (b4_sb, b4_d), (ones_sb, ones_d)]

        xp = ctx.enter_context(tc.tile_pool(name="xsteps", bufs=3))
        sp = ctx.enter_context(tc.tile_pool(name="state", bufs=2))
        sp1 = ctx.enter_context(tc.tile_pool(name="scratch", bufs=1))

        c_prev = None
        h_prev = None
        cT = None

        # Gate bank layout [i | f | o | j]. Each gate gets its OWN PSUM tile
        # (Tile tracks deps at tile granularity — one merged z tile made the
        # first ACT wait for every h-MM and next-step x-MMs wait for every
        # ACT). z_i/z_f are double-buffered so the next step's x-inject into
        # them starts immediately after this step's h-block.
        X_ORDER = (0, 1, 2, 3)     # i, f, o (fresh bufs), then j (after tanh j)
        STOP_ORDER = (3, 1, 0, 2)  # j, f, i, o: tanh(j) early, f's chain is
                                   # the longest, o's sigmoid is last-needed

        def quad_mm(zt, n, lhs_tile, lhs_col, w_k, start, stop):
            ns = slice(n * 512, (n + 1) * 512)
            for j in range(NJ):
                nc.tensor.matmul(
                    zt[32 * j:32 * (j + 1), :],
                    lhs_tile[:, lhs_col + 32 * j: lhs_col + 32 * j + 32],
                    wl_sb[:, j, w_k, ns],
                    start=start, stop=stop,
                    tile_position=(0, 32 * j),
                )

        with tc.tile_pool(name="zi", bufs=2, space="PSUM") as zpi, \
             tc.tile_pool(name="zf", bufs=2, space="PSUM") as zpf, \
             tc.tile_pool(name="zj", bufs=1, space="PSUM") as zpj, \
             tc.tile_pool(name="zo", bufs=2, space="PSUM") as zpo, \
             tc.tile_pool(name="trpsum", bufs=1, space="PSUM") as trpool:
            for t in range(t_steps):
                if t == 2:
                    for sb_t, d_ in _mlp_loads:
                        nc.sync.dma_start(sb_t[:], d_[:])
                xt_sb = xp.tile([128, KX * 128], bf16, tag="xt")
                nc.sync.dma_start(xt_sb[:], xt_d[t])

                z_i = zpi.tile([128, 512], f32, tag="zi", name=f"zi{t % 2}")
                z_f = zpf.tile([128, 512], f32, tag="zf", name=f"zf{t % 2}")
                z_j = zpj.tile([128, 512], f32, tag="zj", name=f"zj{t % 2}")
                z_o = zpo.tile([128, 512], f32, tag="zo", name=f"zo{t % 2}")
                zt = {0: z_i, 1: z_f, 3: z_j, 2: z_o}
                # x-inject: fills the PE while the previous step's tail runs
                for n in X_ORDER:
                    for k in range(KX):
                        quad_mm(zt[n], n, xt_sb, k * 128, k,
                                start=(k == 0), stop=(t == 0 and k == KX - 1))

                if t > 0:
                    # transpose h(t-1) after the x-block in the tensor queue;
                    # the second transpose pair is deferred until after the
                    # k0/k1 sweeps so it never delays the h-block start
                    trp = trpool.tile([128, H], bf16, tag="tr")
                    hT = sp1.tile([128, H], bf16, tag="hT")
                    for kk in (0, 1):
                        ck = slice(kk * 128, (kk + 1) * 128)
                        nc.tensor.transpose(trp[:, ck], h_prev[:, ck],
                                            idr_sb[:])
                    nc.vector.tensor_copy(hT[:, 0:256], trp[:, 0:256])
                    for k in (0, 1):
                        for n in STOP_ORDER:
                            quad_mm(zt[n], n, hT, k * 128, KX + k,
                                    start=False, stop=False)
                    for kk in (2, 3):
                        ck = slice(kk * 128, (kk + 1) * 128)
                        nc.tensor.transpose(trp[:, ck], h_prev[:, ck],
                                            idr_sb[:])
                    nc.vector.tensor_copy(hT[:, 256:512], trp[:, 256:512])
                    for n in STOP_ORDER:
                        for k in (2, 3):
                            quad_mm(zt[n], n, hT, k * 128, KX + k,
                                    start=False, stop=(k == KH - 1))

                # gate ACTs in stop order: tanh(j), sig(f), sig(i), sig(o)
                gates = sp1.tile([128, G4], bf16, tag="gates")
                nc.scalar.activation(gates[:, 1536:2048], zt[3][:], AF.Tanh)
                nc.scalar.activation(gates[:, 512:1024], zt[1][:], AF.Sigmoid)
                nc.scalar.activation(gates[:, 0:512], zt[0][:], AF.Sigmoid)
                nc.scalar.activation(gates[:, 1024:1536], zt[2][:], AF.Sigmoid)

                # tail: c = c_prev*sig(f) + sig(i)*tanh(j); h = tanh(c)*sig(o)
                last = t == t_steps - 1
                c_new = sp.tile([128, H], f32, tag="c")
                if t > 0:
                    t1 = sp1.tile([128, H], f32, tag="t1")
                    t2 = sp1.tile([128, H], bf16, tag="t2")
                if not last:
                    tanc = sp1.tile([128, H], bf16, tag="tanc")
                    h = sp.tile([128, H], bf16, tag="h")

                def gsl(g, hf):
                    base = (0, 512, 1024, 1536)[g]
                    return gates[:, base + hf * 256: base + hf * 256 + 256]

                if t > 0:
                    h0, h1 = slice(0, 256), slice(256, 512)
                    # h1's t1 on GpSimd so the DVE queue stays short on the
                    # h0 critical path
                    nc.gpsimd.tensor_mul(t1[:, h1], c_prev[:, h1], gsl(1, 1))
                    nc.vector.tensor_mul(t1[:, h0], c_prev[:, h0], gsl(1, 0))
                    nc.vector.tensor_mul(t2[:, h0], gsl(0, 0), gsl(3, 0))
                    nc.vector.tensor_add(c_new[:, h0], t1[:, h0], t2[:, h0])
                    nc.vector.tensor_mul(t2[:, h1], gsl(0, 1), gsl(3, 1))
                    nc.vector.tensor_add(c_new[:, h1], t1[:, h1], t2[:, h1])
                else:
                    for hf in (0, 1):
                        sl = slice(hf * 256, (hf + 1) * 256)
                        nc.vector.tensor_mul(c_new[:, sl], gsl(0, hf), gsl(3, hf))
                if not last:
                    for hf in (0, 1):
                        sl = slice(hf * 256, (hf + 1) * 256)
                        nc.scalar.activation(tanc[:, sl], c_new[:, sl], AF.Tanh)
                    for hf in (0, 1):
                        sl = slice(hf * 256, (hf + 1) * 256)
                        nc.vector.tensor_mul(h[:, sl], tanc[:, sl], gsl(2, hf))
                c_prev = c_new
                if not last:
                    h_prev = h
                else:
                    # transpose the final stacked c -> cT; its job-columns
                    # are exactly the rnn-feature chunks the MLP needs
                    cb = sp1.tile([128, H], bf16, tag="cb")
                    nc.vector.tensor_copy(cb[:], c_new[:])
                    trb = trpool.tile([128, H], bf16, tag="tr")
                    for k in range(4):
                        ks = slice(k * 128, (k + 1) * 128)
                        nc.tensor.transpose(trb[:, ks], cb[:, ks], idr_sb[:])
                    cT = sp1.tile([128, H], bf16, tag="cT")
                    nc.vector.tensor_copy(cT[:], trb[:])

        # MLP on this core's 32 rows (activations stationary, weights moving,
        # PE transposes between layers). rnnT chunk kc = cT chunk (kc%4),
        # batch columns of job (kc//4).
        with tc.tile_pool(name="mlppsum", bufs=2, space="PSUM") as mp, \
             tc.tile_pool(name="mtrpsum", bufs=2, space="PSUM") as mtr, \
             tc.tile_pool(name="l4psum", bufs=1, space="PSUM") as mp4:
            def lhs_of(act_T, kc, n_kc):
                if act_T is None:  # layer 1: slices of cT
                    j, k = kc // 4, kc % 4
                    return cT[:, k * 128 + 32 * j: k * 128 + 32 * j + 32]
                return act_T[:, 32 * kc:32 * (kc + 1)]

            act_T = None
            for li, (w_sb, b_sb, n_kc) in enumerate(
                [(w1_sb, b1_sb, 16), (w2_sb, b2_sb, 8), (w3_sb, b3_sb, 8)]
            ):
                aps = mp.tile([BC, F], f32, tag="aps")
                for m2 in (0, 1):
                    ms = slice(m2 * 512, (m2 + 1) * 512)
                    for kc in range(n_kc):
                        nc.tensor.matmul(
                            aps[:, ms], lhs_of(act_T, kc, n_kc),
                            w_sb[:, kc, ms],
                            start=(kc == 0), stop=False,
                        )
                    nc.tensor.matmul(
                        aps[:, ms], ones_sb[0:1, :], b_sb[0:1, ms],
                        start=False, stop=True,
                    )
                a_sb = sp.tile([BC, F], bf16, tag="a")
                nc.scalar.activation(a_sb[:], aps[:], AF.Tanh)
                # transpose a -> aT chunks [128, 32]
                trm = mtr.tile([128, 256], bf16, tag="trm")
                nxt_T = sp.tile([128, 256], bf16, tag="aT")
                for c8 in range(8):
                    nc.tensor.transpose(
                        trm[:, 32 * c8:32 * (c8 + 1)],
                        a_sb[:, 128 * c8:128 * (c8 + 1)],
                        idr_sb[0:BC, 0:BC],
                    )
                nc.vector.tensor_copy(nxt_T[:], trm[:])
                act_T = nxt_T

            l4 = mp4.tile([BC, 3], f32, tag="l4")
            for kc in range(8):
                nc.tensor.matmul(l4[:], act_T[:, 32 * kc:32 * (kc + 1)],
                                 w4_sb[:, kc, :], start=(kc == 0), stop=False)
            nc.tensor.matmul(l4[:], ones_sb[0:1, :], b4_sb[0:1, :],
                             start=False, stop=True)
            lg = sp1.tile([BC, 3], f32, tag="lg")
            nc.scalar.copy(lg[:], l4[:])
            nc.sync.dma_start(out_d[:], lg[:])

    return nc


def _pack_lstm(W, b):
    """Gate-permute [i,f,o,j], fold forget bias, split into 7 chunks."""
    perm = np.concatenate([
        np.arange(0, H), np.arange(2 * H, 3 * H),
        np.arange(3 * H, 4 * H), np.arange(H, 2 * H),
    ])
    Wp = np.asarray(W)[:, perm].astype(np.float32)
    bp = np.asarray(b)[perm].astype(np.float32).copy()
    bp[H:2 * H] += 1.0
    wl = np.zeros((128, KX + KH, 4 * H), np.float32)
    W_aug_x = np.zeros((EP, 4 * H), np.float32)
    W_aug_x[:E] = Wp[:E]
    W_aug_x[E] = bp
    for k in range(KX):
        wl[:, k, :] = W_aug_x[k * 128:(k + 1) * 128]
    for k in range(KH):
        wl[:, KX + k, :] = Wp[E + k * 128: E + (k + 1) * 128]
    return wl


def _pack_shared(inputs, t_steps=T):
    bf16 = ml_dtypes.bfloat16
    wl4 = np.zeros((128, NJ, KX + KH, 4 * H), np.float32)
    wl4[:, 0] = _pack_lstm(inputs["W_fw_p"], inputs["b_fw_p"])
    wl4[:, 1] = _pack_lstm(inputs["W_bw_p"], inputs["b_bw_p"])
    wl4[:, 2] = _pack_lstm(inputs["W_fw_h"], inputs["b_fw_h"])
    wl4[:, 3] = _pack_lstm(inputs["W_bw_h"], inputs["b_bw_h"])

    W1 = np.asarray(inputs["W1"]).astype(np.float32)
    W2 = np.asarray(inputs["W2"]).astype(np.float32)
    W3 = np.asarray(inputs["W3"]).astype(np.float32)
    W4 = np.asarray(inputs["W4"]).astype(np.float32)
    return {
        "wl": wl4.astype(bf16),
        "w1": np.ascontiguousarray(W1.reshape(16, 128, F).transpose(1, 0, 2).astype(bf16)),
        "w2": np.ascontiguousarray(W2.reshape(8, 128, F).transpose(1, 0, 2).astype(bf16)),
        "w3": np.ascontiguousarray(W3.reshape(8, 128, F).transpose(1, 0, 2).astype(bf16)),
        "w4": np.ascontiguousarray(W4.reshape(8, 128, 3).transpose(1, 0, 2).astype(bf16)),
        "b1": np.asarray(inputs["b1"]).reshape(1, F).astype(bf16),
        "b2": np.asarray(inputs["b2"]).reshape(1, F).astype(bf16),
        "b3": np.asarray(inputs["b3"]).reshape(1, F).astype(bf16),
        "b4": np.asarray(inputs["b4"]).reshape(1, 3).astype(bf16),
        "ones": np.ones((1, BC), bf16),
        "identr": np.eye(128, dtype=bf16),
    }


def _pack_core_inputs(core, inputs, shared, t_steps=T):
    """Per-core xt: [t, 128, k*128 + 32*j + b] = x_j[b, t, 128k+p]."""
    bf16 = ml_dtypes.bfloat16
    rows = slice(core * BC, (core + 1) * BC)
    prem = np.asarray(inputs["premises"])[rows, :t_steps]
    hyp = np.asarray(inputs["hypotheses"])[rows, :t_steps]

    xa = np.zeros((NJ, BC, t_steps, EP), np.float32)
    for j, x in enumerate((prem, prem[:, ::-1], hyp, hyp[:, ::-1])):
        xa[j, :, :, :E] = x
        xa[j, :, :, E] = 1.0
    # [NJ, BC, T, KX, 128] -> [T, 128, KX, NJ, BC]
    xt = np.ascontiguousarray(
        xa.reshape(NJ, BC, t_steps, KX, 128).transpose(2, 4, 3, 0, 1)
    ).reshape(t_steps, 128, KX * 128)
    return {"xt": xt.astype(bf16), **shared}


def _install_ntff_shim():
    """This image's `antenv` lacks `axon_hooks`; provide it so
    run_bass_kernel_spmd(trace=True) can capture NTFF profiles."""
    import sys
    import types

    if "antenv.axon_hooks" in sys.modules:
        return
    mod = types.ModuleType("antenv.axon_hooks")
    state = {"hook": None}
    mod.set_axon_ntff_profile_hook = lambda h: state.__setitem__("hook", h)
    mod.get_axon_ntff_profile_hook = lambda: state["hook"]
    sys.modules["antenv.axon_hooks"] = mod
    try:
        from trn_agent_boot.trn_boot import _ntff_profile_via_ctypes

        mod.set_axon_ntff_profile_hook(
            _ntff_profile_via_ctypes("/opt/axon/libaxon_pjrt.so")
        )
    except Exception:
        pass


def _run(inputs, trace=False, t_steps=T, ldt="bf16"):
    if trace:
        _install_ntff_shim()
    from concourse.bass_utils import run_bass_kernel_spmd

    key = (t_steps,)
    if key not in _cache:
        _cache[key] = _build_nc(t_steps)
    nc = _cache[key]
    shared = _pack_shared(inputs, t_steps)
    in_maps = [_pack_core_inputs(c, inputs, shared, t_steps)
               for c in range(N_CORES)]
    res = run_bass_kernel_spmd(
        nc, in_maps, list(range(N_CORES)), trace=trace
    )
    out = np.zeros((B, 3), np.float32)
    for q in range(N_CORES):
        out[q * BC:(q + 1) * BC] = res.results[q]["logits"]
    return out, res


def kernel(**inputs) -> np.ndarray:
    out, _ = _run(inputs, trace=False)
    return out


# revision 14
# speedup vs baseline: 1.2115x; 1.2115x over previous
"""BiRNN (Bowman SNLI) Trainium2 kernel, quad-LSTM column-tiled design.

Sharding: 8 cores = 8 batch slices of 32 rows. Each core runs ALL FOUR
LSTMs (fw_p, bw_p, fw_h, bw_h) for its 32 rows, partition-stacked: job j
occupies batch columns [32j, 32j+32) of every stationary operand and PSUM
rows [32j, 32j+32). Each matmul slot issues 4 concurrent column-tiled MMs
(M=32 at tile_position (0, 32j)) with per-job weights as the moving
operand — the PE runs them in parallel, so the 4 LSTMs cost one LSTM's
matmul time. The elementwise tail (gates/c/h) spans all 4 jobs in single
[128, *] ops, quartering its per-LSTM-step cost vs one-job-per-core.
h^T for step t+1 comes from 4 PE transposes of the stacked h(t), emitted
between step t+1's x-inject MMs and its h-MMs so the tail overlaps the
x-block. No collectives: after the scan each core holds all four final
cell states for its 32 rows and runs the MLP locally (batch-32,
activations-stationary layout with PE transposes between layers).
"""
import numpy as np
import ml_dtypes

# Harness-visible constants
B, T, E, H, F = 256, 128, 300, 512, 1024
BC = 32           # batch rows per core
N_CORES = 8
EP = 384          # padded x feature dim (300 x + 1 bias + pad)
KX = 3            # x stationary chunks (last has 45 valid rows)
KH = 4            # h stationary chunks
NJ = 4            # LSTM jobs per core

_cache = {}


def _apply_tile_patch():
    """walrus here allows ONE semaphore wait per instruction; Tile's tail
    drain (and occasionally other instructions) get more. Split extra waits
    onto same-engine NoOp carriers inserted immediately before."""
    import concourse.tile as tile
    import concourse.mybir as mybir
    from concourse.tile import ScopedClock

    if getattr(tile.TileContext, "_multiwait_patched", False):
        return

    def split_multiwait(nc):
        for f in nc.m.functions:
            for bb in f.blocks:
                insts = bb.instructions
                if not any(
                    i.sync_info is not None and len(i.sync_info.on_wait) > 1
                    for i in insts
                ):
                    continue
                new = []
                for inst in insts:
                    si = inst.sync_info
                    if si is not None and len(si.on_wait) > 1:
                        waits = list(si.on_wait)
                        for w in waits[:-1]:
                            carrier = mybir.InstNoOp(
                                name=nc.get_next_instruction_name(), ins=[], outs=[]
                            )
                            carrier.engine = inst.engine
                            carrier.sync_info = mybir.SyncInfo(
                                on_wait=[w], on_update=[]
                            )
                            nc.register_instruction(carrier, overwrite=True)
                            new.append(carrier)
                        si.on_wait = [waits[-1]]
                    new.append(inst)
                bb.instructions = new

    def _patched_drain_and_barrier(self, tick_clock, wait_clock):
        nc = self.nc
        drain_inst = nc.sync.drain()
        wait_clock.add_sem_waits(
            drain_inst.ins, ScopedClock({None: tick_clock.global_clock})
        )
        nc.all_engine_barrier()
        assert self.sems is not None
        popped = nc._tile_sem_poison_stack.pop()
        assert popped is self._sem_poison
        nc.clear_and_free_semaphores(list(self.sems.allocated().values()))
        nc.all_engine_barrier()
        split_multiwait(nc)

    tile.TileContext._drain_and_barrier = _patched_drain_and_barrier
    tile.TileContext._multiwait_patched = True


def _build_nc(t_steps=T, ldt="bf16"):
    _apply_tile_patch()
    from contextlib import ExitStack
    import concourse.bass as bass
    import concourse.tile as tile
    from concourse import mybir

    f32 = mybir.dt.float32
    bf16 = mybir.dt.bfloat16
    AF = mybir.ActivationFunctionType

    nc = bass.Bass("TRN2", target_bir_lowering=False, debug=False,
                   num_devices=N_CORES)

    xt_d = nc.dram_tensor("xt", [t_steps, 128, KX * 128], bf16, kind="ExternalInput").ap()
    wl_d = nc.dram_tensor("wl", [128, NJ, KX + KH, 4 * H], bf16, kind="ExternalInput").ap()
    w1_d = nc.dram_tensor("w1", [128, 16, F], bf16, kind="ExternalInput").ap()
    w2_d = nc.dram_tensor("w2", [128, 8, F], bf16, kind="ExternalInput").ap()
    w3_d = nc.dram_tensor("w3", [128, 8, F], bf16, kind="ExternalInput").ap()
    w4_d = nc.dram_tensor("w4", [128, 8, 3], bf16, kind="ExternalInput").ap()
    b1_d = nc.dram_tensor("b1", [1, F], bf16, kind="ExternalInput").ap()
    b2_d = nc.dram_tensor("b2", [1, F], bf16, kind="ExternalInput").ap()
    b3_d = nc.dram_tensor("b3", [1, F], bf16, kind="ExternalInput").ap()
    b4_d = nc.dram_tensor("b4", [1, 3], bf16, kind="ExternalInput").ap()
    ones_d = nc.dram_tensor("ones", [1, BC], bf16, kind="ExternalInput").ap()
    idr_d = nc.dram_tensor("identr", [128, 128], bf16, kind="ExternalInput").ap()
    out_d = nc.dram_tensor("logits", [BC, 3], f32, kind="ExternalOutput").ap()

    G4 = 4 * H  # 2048

    with tile.TileContext(nc) as tc, ExitStack() as ctx:
        wp = ctx.enter_context(tc.tile_pool(name="weights", bufs=1))
        wl_sb = wp.tile([128, NJ, KX + KH, G4], bf16, tag="wl")
        w1_sb = wp.tile([128, 16, F], bf16, tag="w1")
        w2_sb = wp.tile([128, 8, F], bf16, tag="w2")
        w3_sb = wp.tile([128, 8, F], bf16, tag="w3")
        w4_sb = wp.tile([128, 8, 3], bf16, tag="w4")
        b1_sb = wp.tile([1, F], bf16, tag="b1")
        b2_sb = wp.tile([1, F], bf16, tag="b2")
        b3_sb = wp.tile([1, F], bf16, tag="b3")
        b4_sb = wp.tile([1, 3], bf16, tag="b4")
        ones_sb = wp.tile([1, BC], bf16, tag="ones")
        idr_sb = wp.tile([128, 128], bf16, tag="idr")
        # LSTM-critical loads first; one DMA per (job, chunk) so the
        # transfers spread across queues and early chunks land first
        nc.sync.dma_start(idr_sb[:], idr_d[:])
        for k in range(KX + KH):
            for j in range(NJ):
                nc.sync.dma_start(wl_sb[:, j, k, :], wl_d[:, j, k])
        _mlp_loads = [(w1_sb, w1_d), (w2_sb, w2_d), (w3_sb, w3_d),
                      (w4_sb, w4_d), (b1_sb, b1_d), (b2_sb, b2_d),
                      (b3_sb, b3_d), (b4_sb, b4_d), (ones_sb, ones_d)]

        xp = ctx.enter_context(tc.tile_pool(name="xsteps", bufs=3))
        sp = ctx.enter_context(tc.tile_pool(name="state", bufs=2))
        sp1 = ctx.enter_context(tc.tile_pool(name="scratch", bufs=1))

        c_prev = None
        h_prev = None
        cT = None

        # Gate bank layout [i | f | o | j]. Each gate gets its OWN PSUM tile
        # (Tile tracks deps at tile granularity — one merged z tile made the
        # first ACT wait for every h-MM and next-step x-MMs wait for every
        # ACT). z_i/z_f are double-buffered so the next step's x-inject into
        # them starts immediately after this step's h-block.
        X_ORDER = (0, 1, 3, 2)     # i, f (fresh bufs), then j, o (after ACTs)
        STOP_ORDER = (3, 1, 0, 2)  # j, f, i, o: tanh(j) early, f's chain is
                                   # the longest, o's sigmoid is last-needed

        def quad_mm(zt, n, lhs_tile, lhs_col, w_k, start, stop):
            ns = slice(n * 512, (n + 1) * 512)
            for j in range(NJ):
                nc.tensor.matmul(
                    zt[32 * j:32 * (j + 1), :],
                    lhs_tile[:, lhs_col + 32 * j: lhs_col + 32 * j + 32],
                    wl_sb[:, j, w_k, ns],
                    start=start, stop=stop,
                    tile_position=(0, 32 * j),
                )

        with tc.tile_pool(name="zi", bufs=2, space="PSUM") as zpi, \
             tc.tile_pool(name="zf", bufs=2, space="PSUM") as zpf, \
             tc.tile_pool(name="zj", bufs=1, space="PSUM") as zpj, \
             tc.tile_pool(name="zo", bufs=1, space="PSUM") as zpo, \
             tc.tile_pool(name="trpsum", bufs=1, space="PSUM") as trpool:
            for t in range(t_steps):
                if t == 2:
                    for sb_t, d_ in _mlp_loads:
                        nc.sync.dma_start(sb_t[:], d_[:])
                xt_sb = xp.tile([128, KX * 128], bf16, tag="xt")
                nc.sync.dma_start(xt_sb[:], xt_d[t])

                z_i = zpi.tile([128, 512], f32, tag="zi", name=f"zi{t % 2}")
                z_f = zpf.tile([128, 512], f32, tag="zf", name=f"zf{t % 2}")
                z_j = zpj.tile([128, 512], f32, tag="zj", name=f"zj{t % 2}")
                z_o = zpo.tile([128, 512], f32, tag="zo", name=f"zo{t % 2}")
                zt = {0: z_i, 1: z_f, 3: z_j, 2: z_o}
                # x-inject: fills the PE while the previous step's tail runs
                for n in X_ORDER:
                    for k in range(KX):
                        quad_mm(zt[n], n, xt_sb, k * 128, k,
                                start=(k == 0), stop=(t == 0 and k == KX - 1))

                if t > 0:
                    # transpose h(t-1) (after the x-block in the tensor
                    # queue), copy PSUM->SBUF, then the h-MMs
                    trp = trpool.tile([128, H], bf16, tag="tr")
                    hT = sp1.tile([128, H], bf16, tag="hT")
                    for hf in (0, 1):
                        for kk in (0, 1):
                            ck = slice(hf * 256 + kk * 128,
                                       hf * 256 + (kk + 1) * 128)
                            nc.tensor.transpose(trp[:, ck], h_prev[:, ck],
                                                idr_sb[:])
                        hs = slice(hf * 256, (hf + 1) * 256)
                        nc.vector.tensor_copy(hT[:, hs], trp[:, hs])
                    # k01 bank-sweeps, then per-bank k23 so the bank stops
                    # spread out and the ACTs pipeline early
                    for k in (0, 1):
                        for n in STOP_ORDER:
                            quad_mm(zt[n], n, hT, k * 128, KX + k,
                                    start=False, stop=False)
                    for n in STOP_ORDER:
                        for k in (2, 3):
                            quad_mm(zt[n], n, hT, k * 128, KX + k,
                                    start=False, stop=(k == KH - 1))

                # gate ACTs in stop order: tanh(j), sig(f), sig(i), sig(o)
                gates = sp1.tile([128, G4], bf16, tag="gates")
                nc.scalar.activation(gates[:, 1536:2048], zt[3][:], AF.Tanh)
                nc.scalar.activation(gates[:, 512:1024], zt[1][:], AF.Sigmoid)
                nc.scalar.activation(gates[:, 0:512], zt[0][:], AF.Sigmoid)
                nc.scalar.activation(gates[:, 1024:1536], zt[2][:], AF.Sigmoid)

                # tail: c = c_prev*sig(f) + sig(i)*tanh(j); h = tanh(c)*sig(o)
                last = t == t_steps - 1
                c_new = sp.tile([128, H], f32, tag="c")
                if t > 0:
                    t1 = sp1.tile([128, H], f32, tag="t1")
                    t2 = sp1.tile([128, H], bf16, tag="t2")
                if not last:
                    tanc = sp1.tile([128, H], bf16, tag="tanc")
                    h = sp.tile([128, H], bf16, tag="h")

                def gsl(g, hf):
                    base = (0, 512, 1024, 1536)[g]
                    return gates[:, base + hf * 256: base + hf * 256 + 256]

                if t > 0:
                    h0, h1 = slice(0, 256), slice(256, 512)
                    # h1's t1 on GpSimd so the DVE queue stays short on the
                    # h0 critical path
                    nc.gpsimd.tensor_mul(t1[:, h1], c_prev[:, h1], gsl(1, 1))
                    nc.vector.tensor_mul(t1[:, h0], c_prev[:, h0], gsl(1, 0))
                    nc.vector.tensor_mul(t2[:, h0], gsl(0, 0), gsl(3, 0))
                    nc.vector.tensor_add(c_new[:, h0], t1[:, h0], t2[:, h0])
                    nc.vector.tensor_mul(t2[:, h1], gsl(0, 1), gsl(3, 1))
                    nc.vector.tensor_add(c_new[:, h1], t1[:, h1], t2[:, h1])
                else:
                    for hf in (0, 1):
                        sl = slice(hf * 256, (hf + 1) * 256)
                        nc.vector.tensor_mul(c_new[:, sl], gsl(0, hf), gsl(3, hf))
                if not last:
                    for hf in (0, 1):
                        sl = slice(hf * 256, (hf + 1) * 256)
                        nc.scalar.activation(tanc[:, sl], c_new[:, sl], AF.Tanh)
                    for hf in (0, 1):
                        sl = slice(hf * 256, (hf + 1) * 256)
                        nc.vector.tensor_mul(h[:, sl], tanc[:, sl], gsl(2, hf))
                c_prev = c_new
                if not last:
                    h_prev = h
                else:
                    # transpose the final stacked c -> cT; its job-columns
                    # are exactly the rnn-feature chunks the MLP needs
                    cb = sp1.tile([128, H], bf16, tag="cb")
                    nc.vector.tensor_copy(cb[:], c_new[:])
                    trb = trpool.tile([128, H], bf16, tag="tr")
                    for k in range(4):
                        ks = slice(k * 128, (k + 1) * 128)
                        nc.tensor.transpose(trb[:, ks], cb[:, ks], idr_sb[:])
                    cT = sp1.tile([128, H], bf16, tag="cT")
                    nc.vector.tensor_copy(cT[:], trb[:])

        # MLP on this core's 32 rows (activations stationary, weights moving,
        # PE transposes between layers). rnnT chunk kc = cT chunk (kc%4),
        # batch columns of job (kc//4).
        with tc.tile_pool(name="mlppsum", bufs=2, space="PSUM") as mp, \
             tc.tile_pool(name="mtrpsum", bufs=2, space="PSUM") as mtr, \
             tc.tile_pool(name="l4psum", bufs=1, space="PSUM") as mp4:
            def lhs_of(act_T, kc, n_kc):
                if act_T is None:  # layer 1: slices of cT
                    j, k = kc // 4, kc % 4
                    return cT[:, k * 128 + 32 * j: k * 128 + 32 * j + 32]
                return act_T[:, 32 * kc:32 * (kc + 1)]

            act_T = None
            for li, (w_sb, b_sb, n_kc) in enumerate(
                [(w1_sb, b1_sb, 16), (w2_sb, b2_sb, 8), (w3_sb, b3_sb, 8)]
            ):
                aps = mp.tile([BC, F], f32, tag="aps")
                for m2 in (0, 1):
                    ms = slice(m2 * 512, (m2 + 1) * 512)
                    for kc in range(n_kc):
                        nc.tensor.matmul(
                            aps[:, ms], lhs_of(act_T, kc, n_kc),
                            w_sb[:, kc, ms],
                            start=(kc == 0), stop=False,
                        )
                    nc.tensor.matmul(
                        aps[:, ms], ones_sb[0:1, :], b_sb[0:1, ms],
                        start=False, stop=True,
                    )
                a_sb = sp.tile([BC, F], bf16, tag="a")
                nc.scalar.activation(a_sb[:], aps[:], AF.Tanh)
                # transpose a -> aT chunks [128, 32]
                trm = mtr.tile([128, 256], bf16, tag="trm")
                nxt_T = sp.tile([128, 256], bf16, tag="aT")
                for c8 in range(8):
                    nc.tensor.transpose(
                        trm[:, 32 * c8:32 * (c8 + 1)],
                        a_sb[:, 128 * c8:128 * (c8 + 1)],
                        idr_sb[0:BC, 0:BC],
                    )
                nc.vector.tensor_copy(nxt_T[:], trm[:])
                act_T = nxt_T

            l4 = mp4.tile([BC, 3], f32, tag="l4")
            for kc in range(8):
                nc.tensor.matmul(l4[:], act_T[:, 32 * kc:32 * (kc + 1)],
                                 w4_sb[:, kc, :], start=(kc == 0), stop=False)
            nc.tensor.matmul(l4[:], ones_sb[0:1, :], b4_sb[0:1, :],
                             start=False, stop=True)
            lg = sp1.tile([BC, 3], f32, tag="lg")
            nc.scalar.copy(lg[:], l4[:])
            nc.sync.dma_start(out_d[:], lg[:])

    return nc


def _pack_lstm(W, b):
    """Gate-permute [i,f,o,j], fold forget bias, split into 7 chunks."""
    perm = np.concatenate([
        np.arange(0, H), np.arange(2 * H, 3 * H),
        np.arange(3 * H, 4 * H), np.arange(H, 2 * H),
    ])
    Wp = np.asarray(W)[:, perm].astype(np.float32)
    bp = np.asarray(b)[perm].astype(np.float32).copy()
    bp[H:2 * H] += 1.0
    wl = np.zeros((128, KX + KH, 4 * H), np.float32)
    W_aug_x = np.zeros((EP, 4 * H), np.float32)
    W_aug_x[:E] = Wp[:E]
    W_aug_x[E] = bp
    for k in range(KX):
        wl[:, k, :] = W_aug_x[k * 128:(k + 1) * 128]
    for k in range(KH):
        wl[:, KX + k, :] = Wp[E + k * 128: E + (k + 1) * 128]
    return wl


def _pack_shared(inputs, t_steps=T):
    bf16 = ml_dtypes.bfloat16
    wl4 = np.zeros((128, NJ, KX + KH, 4 * H), np.float32)
    wl4[:, 0] = _pack_lstm(inputs["W_fw_p"], inputs["b_fw_p"])
    wl4[:, 1] = _pack_lstm(inputs["W_bw_p"], inputs["b_bw_p"])
    wl4[:, 2] = _pack_lstm(inputs["W_fw_h"], inputs["b_fw_h"])
    wl4[:, 3] = _pack_lstm(inputs["W_bw_h"], inputs["b_bw_h"])

    W1 = np.asarray(inputs["W1"]).astype(np.float32)
    W2 = np.asarray(inputs["W2"]).astype(np.float32)
    W3 = np.asarray(inputs["W3"]).astype(np.float32)
    W4 = np.asarray(inputs["W4"]).astype(np.float32)
    return {
        "wl": wl4.astype(bf16),
        "w1": np.ascontiguousarray(W1.reshape(16, 128, F).transpose(1, 0, 2).astype(bf16)),
        "w2": np.ascontiguousarray(W2.reshape(8, 128, F).transpose(1, 0, 2).astype(bf16)),
        "w3": np.ascontiguousarray(W3.reshape(8, 128, F).transpose(1, 0, 2).astype(bf16)),
        "w4": np.ascontiguousarray(W4.reshape(8, 128, 3).transpose(1, 0, 2).astype(bf16)),
        "b1": np.asarray(inputs["b1"]).reshape(1, F).astype(bf16),
        "b2": np.asarray(inputs["b2"]).reshape(1, F).astype(bf16),
        "b3": np.asarray(inputs["b3"]).reshape(1, F).astype(bf16),
        "b4": np.asarray(inputs["b4"]).reshape(1, 3).astype(bf16),
        "ones": np.ones((1, BC), bf16),
        "identr": np.eye(128, dtype=bf16),
    }


def _pack_core_inputs(core, inputs, shared, t_steps=T):
    """Per-core xt: [t, 128, k*128 + 32*j + b] = x_j[b, t, 128k+p]."""
    bf16 = ml_dtypes.bfloat16
    rows = slice(core * BC, (core + 1) * BC)
    prem = np.asarray(inputs["premises"])[rows, :t_steps]
    hyp = np.asarray(inputs["hypotheses"])[rows, :t_steps]

    xa = np.zeros((NJ, BC, t_steps, EP), np.float32)
    for j, x in enumerate((prem, prem[:, ::-1], hyp, hyp[:, ::-1])):
        xa[j, :, :, :E] = x
        xa[j, :, :, E] = 1.0
    # [NJ, BC, T, KX, 128] -> [T, 128, KX, NJ, BC]
    xt = np.ascontiguousarray(
        xa.reshape(NJ, BC, t_steps, KX, 128).transpose(2, 4, 3, 0, 1)
    ).reshape(t_steps, 128, KX * 128)
    return {"xt": xt.astype(bf16), **shared}


def _install_ntff_shim():
    """This image's `antenv` lacks `axon_hooks`; provide it so
    run_bass_kernel_spmd(trace=True) can capture NTFF profiles."""
    import sys
    import types

    if "antenv.axon_hooks" in sys.modules:
        return
    mod = types.ModuleType("antenv.axon_hooks")
    state = {"hook": None}
    mod.set_axon_ntff_profile_hook = lambda h: state.__setitem__("hook", h)
    mod.get_axon_ntff_profile_hook = lambda: state["hook"]
    sys.modules["antenv.axon_hooks"] = mod
    try:
        from trn_agent_boot.trn_boot import _ntff_profile_via_ctypes

        mod.set_axon_ntff_profile_hook(
            _ntff_profile_via_ctypes("/opt/axon/libaxon_pjrt.so")
        )
    except Exception:
        pass


def _run(inputs, trace=False, t_steps=T, ldt="bf16"):
    if trace:
        _install_ntff_shim()
    from concourse.bass_utils import run_bass_kernel_spmd

    key = (t_steps,)
    if key not in _cache:
        _cache[key] = _build_nc(t_steps)
    nc = _cache[key]
    shared = _pack_shared(inputs, t_steps)
    in_maps = [_pack_core_inputs(c, inputs, shared, t_steps)
               for c in range(N_CORES)]
    res = run_bass_kernel_spmd(
        nc, in_maps, list(range(N_CORES)), trace=trace
    )
    out = np.zeros((B, 3), np.float32)
    for q in range(N_CORES):
        out[q * BC:(q + 1) * BC] = res.results[q]["logits"]
    return out, res


def kernel(**inputs) -> np.ndarray:
    out, _ = _run(inputs, trace=False)
    return out


# revision 16
# speedup vs baseline: 1.2123x; 1.0007x over previous
"""BiRNN (Bowman SNLI) Trainium2 kernel, quad-LSTM column-tiled design.

Sharding: 8 cores = 8 batch slices of 32 rows. Each core runs ALL FOUR
LSTMs (fw_p, bw_p, fw_h, bw_h) for its 32 rows, partition-stacked: job j
occupies batch columns [32j, 32j+32) of every stationary operand and PSUM
rows [32j, 32j+32). Each matmul slot issues 4 concurrent column-tiled MMs
(M=32 at tile_position (0, 32j)) with per-job weights as the moving
operand — the PE runs them in parallel, so the 4 LSTMs cost one LSTM's
matmul time. The elementwise tail (gates/c/h) spans all 4 jobs in single
[128, *] ops, quartering its per-LSTM-step cost vs one-job-per-core.
h^T for step t+1 comes from 4 PE transposes of the stacked h(t), emitted
between step t+1's x-inject MMs and its h-MMs so the tail overlaps the
x-block. No collectives: after the scan each core holds all four final
cell states for its 32 rows and runs the MLP locally (batch-32,
activations-stationary layout with PE transposes between layers).
"""
import numpy as np
import ml_dtypes

# Harness-visible constants
B, T, E, H, F = 256, 128, 300, 512, 1024
BC = 32           # batch rows per core
N_CORES = 8
EP = 384          # padded x feature dim (300 x + 1 bias + pad)
KX = 3            # x stationary chunks (last has 45 valid rows)
KH = 4            # h stationary chunks
NJ = 4            # LSTM jobs per core

_cache = {}


def _apply_tile_patch():
    """walrus here allows ONE semaphore wait per instruction; Tile's tail
    drain (and occasionally other instructions) get more. Split extra waits
    onto same-engine NoOp carriers inserted immediately before."""
    import concourse.tile as tile
    import concourse.mybir as mybir
    from concourse.tile import ScopedClock

    if getattr(tile.TileContext, "_multiwait_patched", False):
        return

    def split_multiwait(nc):
        for f in nc.m.functions:
            for bb in f.blocks:
                insts = bb.instructions
                if not any(
                    i.sync_info is not None and len(i.sync_info.on_wait) > 1
                    for i in insts
                ):
                    continue
                new = []
                for inst in insts:
                    si = inst.sync_info
                    if si is not None and len(si.on_wait) > 1:
                        waits = list(si.on_wait)
                        for w in waits[:-1]:
                            carrier = mybir.InstNoOp(
                                name=nc.get_next_instruction_name(), ins=[], outs=[]
                            )
                            carrier.engine = inst.engine
                            carrier.sync_info = mybir.SyncInfo(
                                on_wait=[w], on_update=[]
                            )
                            nc.register_instruction(carrier, overwrite=True)
                            new.append(carrier)
                        si.on_wait = [waits[-1]]
                    new.append(inst)
                bb.instructions = new

    def _patched_drain_and_barrier(self, tick_clock, wait_clock):
        nc = self.nc
        drain_inst = nc.sync.drain()
        wait_clock.add_sem_waits(
            drain_inst.ins, ScopedClock({None: tick_clock.global_clock})
        )
        nc.all_engine_barrier()
        assert self.sems is not None
        popped = nc._tile_sem_poison_stack.pop()
        assert popped is self._sem_poison
        nc.clear_and_free_semaphores(list(self.sems.allocated().values()))
        nc.all_engine_barrier()
        split_multiwait(nc)

    tile.TileContext._drain_and_barrier = _patched_drain_and_barrier
    tile.TileContext._multiwait_patched = True


def _build_nc(t_steps=T, ldt="bf16"):
    _apply_tile_patch()
    from contextlib import ExitStack
    import concourse.bass as bass
    import concourse.tile as tile
    from concourse import mybir

    f32 = mybir.dt.float32
    bf16 = mybir.dt.bfloat16
    AF = mybir.ActivationFunctionType

    nc = bass.Bass("TRN2", target_bir_lowering=False, debug=False,
                   num_devices=N_CORES)

    xt_d = nc.dram_tensor("xt", [t_steps, 128, KX * 128], bf16, kind="ExternalInput").ap()
    wl_d = nc.dram_tensor("wl", [128, NJ, KX + KH, 4 * H], bf16, kind="ExternalInput").ap()
    w1_d = nc.dram_tensor("w1", [128, 16, F], bf16, kind="ExternalInput").ap()
    w2_d = nc.dram_tensor("w2", [128, 8, F], bf16, kind="ExternalInput").ap()
    w3_d = nc.dram_tensor("w3", [128, 8, F], bf16, kind="ExternalInput").ap()
    w4_d = nc.dram_tensor("w4", [128, 8, 3], bf16, kind="ExternalInput").ap()
    b1_d = nc.dram_tensor("b1", [1, F], bf16, kind="ExternalInput").ap()
    b2_d = nc.dram_tensor("b2", [1, F], bf16, kind="ExternalInput").ap()
    b3_d = nc.dram_tensor("b3", [1, F], bf16, kind="ExternalInput").ap()
    b4_d = nc.dram_tensor("b4", [1, 3], bf16, kind="ExternalInput").ap()
    ones_d = nc.dram_tensor("ones", [1, BC], bf16, kind="ExternalInput").ap()
    idr_d = nc.dram_tensor("identr", [128, 128], bf16, kind="ExternalInput").ap()
    out_d = nc.dram_tensor("logits", [BC, 3], f32, kind="ExternalOutput").ap()

    G4 = 4 * H  # 2048

    with tile.TileContext(nc) as tc, ExitStack() as ctx:
        wp = ctx.enter_context(tc.tile_pool(name="weights", bufs=1))
        wl_sb = wp.tile([128, NJ, KX + KH, G4], bf16, tag="wl")
        w1_sb = wp.tile([128, 16, F], bf16, tag="w1")
        w2_sb = wp.tile([128, 8, F], bf16, tag="w2")
        w3_sb = wp.tile([128, 8, F], bf16, tag="w3")
        w4_sb = wp.tile([128, 8, 3], bf16, tag="w4")
        b1_sb = wp.tile([1, F], bf16, tag="b1")
        b2_sb = wp.tile([1, F], bf16, tag="b2")
        b3_sb = wp.tile([1, F], bf16, tag="b3")
        b4_sb = wp.tile([1, 3], bf16, tag="b4")
        ones_sb = wp.tile([1, BC], bf16, tag="ones")
        idr_sb = wp.tile([128, 128], bf16, tag="idr")
        # LSTM-critical loads first; one DMA per (job, chunk) so the
        # transfers spread across queues and early chunks land first
        nc.sync.dma_start(idr_sb[:], idr_d[:])
        for k in range(KX + KH):
            for j in range(NJ):
                nc.sync.dma_start(wl_sb[:, j, k, :], wl_d[:, j, k])
        _mlp_loads = [(w1_sb, w1_d), (w2_sb, w2_d), (w3_sb, w3_d),
                      (w4_sb, w4_d), (b1_sb, b1_d), (b2_sb, b2_d),
                      (b3_sb, b3_d), (b4_sb, b4_d), (ones_sb, ones_d)]

        xp = ctx.enter_context(tc.tile_pool(name="xsteps", bufs=3))
        sp = ctx.enter_context(tc.tile_pool(name="state", bufs=2))
        sp1 = ctx.enter_context(tc.tile_pool(name="scratch", bufs=1))

        c_prev = None
        h_prev = None
        cT = None

        # Gate bank layout [i | f | o | j]. Each gate gets its OWN PSUM tile
        # (Tile tracks deps at tile granularity — one merged z tile made the
        # first ACT wait for every h-MM and next-step x-MMs wait for every
        # ACT). z_i/z_f are double-buffered so the next step's x-inject into
        # them starts immediately after this step's h-block.
        X_ORDER = (0, 1, 3, 2)     # i, f (fresh bufs), then j, o (after ACTs)
        STOP_ORDER = (3, 1, 0, 2)  # j, f, i, o: tanh(j) early, f's chain is
                                   # the longest, o's sigmoid is last-needed

        def quad_mm(zt, n, lhs_tile, lhs_col, w_k, start, stop):
            ns = slice(n * 512, (n + 1) * 512)
            for j in range(NJ):
                nc.tensor.matmul(
                    zt[32 * j:32 * (j + 1), :],
                    lhs_tile[:, lhs_col + 32 * j: lhs_col + 32 * j + 32],
                    wl_sb[:, j, w_k, ns],
                    start=start, stop=stop,
                    tile_position=(0, 32 * j),
                )

        with tc.tile_pool(name="zi", bufs=2, space="PSUM") as zpi, \
             tc.tile_pool(name="zf", bufs=2, space="PSUM") as zpf, \
             tc.tile_pool(name="zj", bufs=1, space="PSUM") as zpj, \
             tc.tile_pool(name="zo", bufs=1, space="PSUM") as zpo, \
             tc.tile_pool(name="trpsum", bufs=1, space="PSUM") as trpool:
            for t in range(t_steps):
                if t == 2:
                    for sb_t, d_ in _mlp_loads:
                        nc.sync.dma_start(sb_t[:], d_[:])
                xt_sb = xp.tile([128, KX * 128], bf16, tag="xt")
                nc.sync.dma_start(xt_sb[:], xt_d[t])

                z_i = zpi.tile([128, 512], f32, tag="zi", name=f"zi{t % 2}")
                z_f = zpf.tile([128, 512], f32, tag="zf", name=f"zf{t % 2}")
                z_j = zpj.tile([128, 512], f32, tag="zj", name=f"zj{t % 2}")
                z_o = zpo.tile([128, 512], f32, tag="zo", name=f"zo{t % 2}")
                zt = {0: z_i, 1: z_f, 3: z_j, 2: z_o}
                # x-inject: fills the PE while the previous step's tail runs
                for n in X_ORDER:
                    for k in range(KX):
                        quad_mm(zt[n], n, xt_sb, k * 128, k,
                                start=(k == 0), stop=(t == 0 and k == KX - 1))

                if t > 0:
                    # transpose h(t-1) (after the x-block in the tensor
                    # queue), copy PSUM->SBUF, then the h-MMs
                    trp = trpool.tile([128, H], bf16, tag="tr")
                    hT = sp1.tile([128, H], bf16, tag="hT")
                    for hf in (0, 1):
                        for kk in (0, 1):
                            ck = slice(hf * 256 + kk * 128,
                                       hf * 256 + (kk + 1) * 128)
                            nc.tensor.transpose(trp[:, ck], h_prev[:, ck],
                                                idr_sb[:])
                        hs = slice(hf * 256, (hf + 1) * 256)
                        nc.vector.tensor_copy(hT[:, hs], trp[:, hs])
                    # k01 bank-sweeps, then per-bank k23 so the bank stops
                    # spread out and the ACTs pipeline early
                    for k in (0, 1):
                        for n in STOP_ORDER:
                            quad_mm(zt[n], n, hT, k * 128, KX + k,
                                    start=False, stop=False)
                    for n in STOP_ORDER:
                        for k in (2, 3):
                            quad_mm(zt[n], n, hT, k * 128, KX + k,
                                    start=False, stop=(k == KH - 1))

                # gate ACTs in stop order: tanh(j), sig(f), sig(i), sig(o).
                # Per-gate SBUF tiles: a single merged gates tile made every
                # ACT write serialize (tile-granular) against every tail read
                # — the trace showed sig(o) statically scheduled ~1.5us late
                # behind a GpSimd read it doesn't actually depend on.
                g_i = sp1.tile([128, 512], bf16, tag="gi")
                g_f = sp1.tile([128, 512], bf16, tag="gf")
                g_o = sp1.tile([128, 512], bf16, tag="go")
                g_j = sp1.tile([128, 512], bf16, tag="gj")
                g_tiles = (g_i, g_f, g_o, g_j)
                nc.scalar.activation(g_j[:], zt[3][:], AF.Tanh)
                nc.scalar.activation(g_f[:], zt[1][:], AF.Sigmoid)
                nc.scalar.activation(g_i[:], zt[0][:], AF.Sigmoid)
                nc.scalar.activation(g_o[:], zt[2][:], AF.Sigmoid)

                # tail: c = c_prev*sig(f) + sig(i)*tanh(j); h = tanh(c)*sig(o)
                last = t == t_steps - 1
                c_new = sp.tile([128, H], f32, tag="c")
                if t > 0:
                    t1 = sp1.tile([128, H], f32, tag="t1")
                    t2 = sp1.tile([128, H], bf16, tag="t2")
                if not last:
                    tanc = sp1.tile([128, H], bf16, tag="tanc")
                    h = sp.tile([128, H], bf16, tag="h")

                def gsl(g, hf):
                    return g_tiles[g][:, hf * 256: hf * 256 + 256]

                if t > 0:
                    h0, h1 = slice(0, 256), slice(256, 512)
                    # h1's t1 on GpSimd so the DVE queue stays short on the
                    # h0 critical path
                    nc.gpsimd.tensor_mul(t1[:, h1], c_prev[:, h1], gsl(1, 1))
                    nc.vector.tensor_mul(t1[:, h0], c_prev[:, h0], gsl(1, 0))
                    nc.vector.tensor_mul(t2[:, h0], gsl(0, 0), gsl(3, 0))
                    nc.vector.tensor_add(c_new[:, h0], t1[:, h0], t2[:, h0])
                    nc.vector.tensor_mul(t2[:, h1], gsl(0, 1), gsl(3, 1))
                    nc.vector.tensor_add(c_new[:, h1], t1[:, h1], t2[:, h1])
                else:
                    for hf in (0, 1):
                        sl = slice(hf * 256, (hf + 1) * 256)
                        nc.vector.tensor_mul(c_new[:, sl], gsl(0, hf), gsl(3, hf))
                if not last:
                    for hf in (0, 1):
                        sl = slice(hf * 256, (hf + 1) * 256)
                        nc.scalar.activation(tanc[:, sl], c_new[:, sl], AF.Tanh)
                    for hf in (0, 1):
                        sl = slice(hf * 256, (hf + 1) * 256)
                        nc.vector.tensor_mul(h[:, sl], tanc[:, sl], gsl(2, hf))
                c_prev = c_new
                if not last:
                    h_prev = h
                else:
                    # transpose the final stacked c -> cT; its job-columns
                    # are exactly the rnn-feature chunks the MLP needs
                    cb = sp1.tile([128, H], bf16, tag="cb")
                    nc.vector.tensor_copy(cb[:], c_new[:])
                    trb = trpool.tile([128, H], bf16, tag="tr")
                    for k in range(4):
                        ks = slice(k * 128, (k + 1) * 128)
                        nc.tensor.transpose(trb[:, ks], cb[:, ks], idr_sb[:])
                    cT = sp1.tile([128, H], bf16, tag="cT")
                    nc.vector.tensor_copy(cT[:], trb[:])

        # MLP on this core's 32 rows (activations stationary, weights moving,
        # PE transposes between layers). rnnT chunk kc = cT chunk (kc%4),
        # batch columns of job (kc//4).
        with tc.tile_pool(name="mlppsum", bufs=2, space="PSUM") as mp, \
             tc.tile_pool(name="mtrpsum", bufs=2, space="PSUM") as mtr, \
             tc.tile_pool(name="l4psum", bufs=1, space="PSUM") as mp4:
            def lhs_of(act_T, kc, n_kc):
                if act_T is None:  # layer 1: slices of cT
                    j, k = kc // 4, kc % 4
                    return cT[:, k * 128 + 32 * j: k * 128 + 32 * j + 32]
                return act_T[:, 32 * kc:32 * (kc + 1)]

            act_T = None
            for li, (w_sb, b_sb, n_kc) in enumerate(
                [(w1_sb, b1_sb, 16), (w2_sb, b2_sb, 8), (w3_sb, b3_sb, 8)]
            ):
                aps = mp.tile([BC, F], f32, tag="aps")
                for m2 in (0, 1):
                    ms = slice(m2 * 512, (m2 + 1) * 512)
                    for kc in range(n_kc):
                        nc.tensor.matmul(
                            aps[:, ms], lhs_of(act_T, kc, n_kc),
                            w_sb[:, kc, ms],
                            start=(kc == 0), stop=False,
                        )
                    nc.tensor.matmul(
                        aps[:, ms], ones_sb[0:1, :], b_sb[0:1, ms],
                        start=False, stop=True,
                    )
                a_sb = sp.tile([BC, F], bf16, tag="a")
                nc.scalar.activation(a_sb[:], aps[:], AF.Tanh)
                # transpose a -> aT chunks [128, 32]
                trm = mtr.tile([128, 256], bf16, tag="trm")
                nxt_T = sp.tile([128, 256], bf16, tag="aT")
                for c8 in range(8):
                    nc.tensor.transpose(
                        trm[:, 32 * c8:32 * (c8 + 1)],
                        a_sb[:, 128 * c8:128 * (c8 + 1)],
                        idr_sb[0:BC, 0:BC],
                    )
                nc.vector.tensor_copy(nxt_T[:], trm[:])
                act_T = nxt_T

            l4 = mp4.tile([BC, 3], f32, tag="l4")
            for kc in range(8):
                nc.tensor.matmul(l4[:], act_T[:, 32 * kc:32 * (kc + 1)],
                                 w4_sb[:, kc, :], start=(kc == 0), stop=False)
            nc.tensor.matmul(l4[:], ones_sb[0:1, :], b4_sb[0:1, :],
                             start=False, stop=True)
            lg = sp1.tile([BC, 3], f32, tag="lg")
            nc.scalar.copy(lg[:], l4[:])
            nc.sync.dma_start(out_d[:], lg[:])

    return nc


def _pack_lstm(W, b):
    """Gate-permute [i,f,o,j], fold forget bias, split into 7 chunks."""
    perm = np.concatenate([
        np.arange(0, H), np.arange(2 * H, 3 * H),
        np.arange(3 * H, 4 * H), np.arange(H, 2 * H),
    ])
    Wp = np.asarray(W)[:, perm].astype(np.float32)
    bp = np.asarray(b)[perm].astype(np.float32).copy()
    bp[H:2 * H] += 1.0
    wl = np.zeros((128, KX + KH, 4 * H), np.float32)
    W_aug_x = np.zeros((EP, 4 * H), np.float32)
    W_aug_x[:E] = Wp[:E]
    W_aug_x[E] = bp
    for k in range(KX):
        wl[:, k, :] = W_aug_x[k * 128:(k + 1) * 128]
    for k in range(KH):
        wl[:, KX + k, :] = Wp[E + k * 128: E + (k + 1) * 128]
    return wl


def _pack_shared(inputs, t_steps=T):
    bf16 = ml_dtypes.bfloat16
    wl4 = np.zeros((128, NJ, KX + KH, 4 * H), np.float32)
    wl4[:, 0] = _pack_lstm(inputs["W_fw_p"], inputs["b_fw_p"])
    wl4[:, 1] = _pack_lstm(inputs["W_bw_p"], inputs["b_bw_p"])
    wl4[:, 2] = _pack_lstm(inputs["W_fw_h"], inputs["b_fw_h"])
    wl4[:, 3] = _pack_lstm(inputs["W_bw_h"], inputs["b_bw_h"])

    W1 = np.asarray(inputs["W1"]).astype(np.float32)
    W2 = np.asarray(inputs["W2"]).astype(np.float32)
    W3 = np.asarray(inputs["W3"]).astype(np.float32)
    W4 = np.asarray(inputs["W4"]).astype(np.float32)
    return {
        "wl": wl4.astype(bf16),
        "w1": np.ascontiguousarray(W1.reshape(16, 128, F).transpose(1, 0, 2).astype(bf16)),
        "w2": np.ascontiguousarray(W2.reshape(8, 128, F).transpose(1, 0, 2).astype(bf16)),
        "w3": np.ascontiguousarray(W3.reshape(8, 128, F).transpose(1, 0, 2).astype(bf16)),
        "w4": np.ascontiguousarray(W4.reshape(8, 128, 3).transpose(1, 0, 2).astype(bf16)),
        "b1": np.asarray(inputs["b1"]).reshape(1, F).astype(bf16),
        "b2": np.asarray(inputs["b2"]).reshape(1, F).astype(bf16),
        "b3": np.asarray(inputs["b3"]).reshape(1, F).astype(bf16),
        "b4": np.asarray(inputs["b4"]).reshape(1, 3).astype(bf16),
        "ones": np.ones((1, BC), bf16),
        "identr": np.eye(128, dtype=bf16),
    }


def _pack_core_inputs(core, inputs, shared, t_steps=T):
    """Per-core xt: [t, 128, k*128 + 32*j + b] = x_j[b, t, 128k+p]."""
    bf16 = ml_dtypes.bfloat16
    rows = slice(core * BC, (core + 1) * BC)
    prem = np.asarray(inputs["premises"])[rows, :t_steps]
    hyp = np.asarray(inputs["hypotheses"])[rows, :t_steps]

    xa = np.zeros((NJ, BC, t_steps, EP), np.float32)
    for j, x in enumerate((prem, prem[:, ::-1], hyp, hyp[:, ::-1])):
        xa[j, :, :, :E] = x
        xa[j, :, :, E] = 1.0
    # [NJ, BC, T, KX, 128] -> [T, 128, KX, NJ, BC]
    xt = np.ascontiguousarray(
        xa.reshape(NJ, BC, t_steps, KX, 128).transpose(2, 4, 3, 0, 1)
    ).reshape(t_steps, 128, KX * 128)
    return {"xt": xt.astype(bf16), **shared}


def _install_ntff_shim():
    """This image's `antenv` lacks `axon_hooks`; provide it so
    run_bass_kernel_spmd(trace=True) can capture NTFF profiles."""
    import sys
    import types

    if "antenv.axon_hooks" in sys.modules:
        return
    mod = types.ModuleType("antenv.axon_hooks")
    state = {"hook": None}
    mod.set_axon_ntff_profile_hook = lambda h: state.__setitem__("hook", h)
    mod.get_axon_ntff_profile_hook = lambda: state["hook"]
    sys.modules["antenv.axon_hooks"] = mod
    try:
        from trn_agent_boot.trn_boot import _ntff_profile_via_ctypes

        mod.set_axon_ntff_profile_hook(
            _ntff_profile_via_ctypes("/opt/axon/libaxon_pjrt.so")
        )
    except Exception:
        pass


def _run(inputs, trace=False, t_steps=T, ldt="bf16"):
    if trace:
        _install_ntff_shim()
    from concourse.bass_utils import run_bass_kernel_spmd

    key = (t_steps,)
    if key not in _cache:
        _cache[key] = _build_nc(t_steps)
    nc = _cache[key]
    shared = _pack_shared(inputs, t_steps)
    in_maps = [_pack_core_inputs(c, inputs, shared, t_steps)
               for c in range(N_CORES)]
    res = run_bass_kernel_spmd(
        nc, in_maps, list(range(N_CORES)), trace=trace
    )
    out = np.zeros((B, 3), np.float32)
    for q in range(N_CORES):
        out[q * BC:(q + 1) * BC] = res.results[q]["logits"]
    return out, res


def kernel(**inputs) -> np.ndarray:
    out, _ = _run(inputs, trace=False)
    return out
